# revision 38
# baseline (speedup 1.0000x reference)
"""GAT additive-attention kernel (nn_GAT) for 8 Trainium2 NeuronCores.

reference:
    k = x @ w_k; q = x @ w_q                      # [bz, N, 1]
    s[b,i,j]   = leaky_relu(k[b,i] + q[b,j], 0.2)
    attn       = softmax(s, axis=j)
    out        = (attn @ x).transpose(0, 2, 1)    # [bz, F, N]

Key identity: with sigma[i,j] = sign(k_i + q_j),
    exp(lrelu(s)) = exp(k_i)exp(q_j)       if s > 0
                  = exp(.2 k_i)exp(.2 q_j) if s <= 0
so with U = exp(q_j - S)*[x_j|1], V = exp(.2(q_j - S))*[x_j|1]:
    out_i = num_i / den_i,  [num|den]_i = (M@U)_i + e_i * (colsum(V) - (M@V))_i
where M = (sigma+1)/2 and e_i = exp(-.8 (k_i + S)).  Everything reduces
to ONE N x N masked matmul  T = sigma-ish @ [U|V]  (fp16) plus O(N*F)
pre/post work.

v5: the exponent shift S is a FIXED constant (fp16 is floating point, so
any S with max(q)-S <= ~10 keeps the top softmax weights in range and only
flushes weights ~e^-17 below the row top — far below the error budget).
This removes the global max(q) reduction from the critical path entirely:
q, exp(q), W and the gamma column sums all stream chunk-by-chunk behind
the x DMA, so the masked-matmul stream starts as soon as the first x
quarter lands.  k_bcast comes from one matmul stage (wk replicated as the
stationary operand against x^T) into four SEPARATE psum tiles (a shared
tile serializes matmul g+1 behind the evacuation of g via a whole-tile
WAR hazard).  The k dot-products (only needed for e in the post phase)
are gated behind eq via a dummy write so the scheduler cannot hoist them
into the critical window.  The G correction add is folded into the PSUM
evacuation, and the post phase runs in two pieces so the first out-DMA
overlaps the second piece's compute.

All 16 row-chunk accumulators live in PSUM simultaneously by packing
2-3 accumulation groups per bank: start=True (which clears the WHOLE
bank's has_written bits) is only used by the first group touching each
bank; later groups open with start=False, which overwrites where the
bits are clear and accumulates afterwards.

Sharding: core c handles batch b = c//2, row-half h = c%2 (2048 rows),
fully data-parallel (no collectives).
"""

import sys
import numpy as np

for _p in ("/opt/trn_rl_repo",):
    if _p not in sys.path:
        sys.path.insert(0, _p)

N = 4096
F = 64
BZ = 4
HALF = 2048
NCH = 32          # j-chunks of 128 (full N)
NIC = 16          # i-chunks of 128 (this core's half)
NEG_SLOPE = 0.2
SLOT_OFF = 176    # fp32 elems between accumulation groups within a bank
SHIFT = 40.0      # fixed exponent shift; safe while max(x @ w_q) < 50

# sign-mask chunks on the ACT engine (slot A).  The first chunks stay on the
# DVE so the matmul stream can start as soon as W chunk 0 exists; ACT gets 8
# mid-pattern chunks, which is all it can sustain alongside its other work.
ACTSET = frozenset(c for c in range(NCH) if c % 8 in (3, 4, 5))
LAST_ACT = max(ACTSET)

_CACHE = {}


def _body(nc, tc):
    import dataclasses
    import concourse.mybir as mybir

    f32 = mybir.dt.float32
    f16 = mybir.dt.float16
    bf16 = mybir.dt.bfloat16
    Alu = mybir.AluOpType
    Act = mybir.ActivationFunctionType

    # host-prepacked inputs (see make_in_maps); x ships as f16 to halve the
    # input DMA bytes (W is built in f16 anyway; q/k dots accumulate in fp32)
    xfp_d = nc.dram_tensor("xfp", [128, NCH * F], f16, kind="ExternalInput").ap()
    xhp_d = nc.dram_tensor("xhp", [128, NIC * F], f16, kind="ExternalInput").ap()
    xht_d = nc.dram_tensor("xht", [64, HALF], bf16, kind="ExternalInput").ap()
    wkb_d = nc.dram_tensor("wkb", [64, 128], bf16, kind="ExternalInput").ap()
    wqkh_d = nc.dram_tensor("wqkh", [128, 128], f16, kind="ExternalInput").ap()
    out_d = nc.dram_tensor("out", [128, NIC * F], f32, kind="ExternalOutput").ap()

    with (
        tc.tile_pool(name="const", bufs=1) as cp,
        tc.tile_pool(name="sb", bufs=1) as sp,
        tc.tile_pool(name="maskp", bufs=8) as mp,
    ):
        ones_row16 = cp.tile([1, 128], f16)
        nc.gpsimd.memset(ones_row16[:], 1.0)
        ones_col16 = cp.tile([128, 1], f16)
        nc.gpsimd.memset(ones_col16[:], 1.0)
        bias_q = cp.tile([128, 1], f32)
        nc.gpsimd.memset(bias_q[:], -SHIFT)
        bias_q2 = cp.tile([128, 1], f32)
        nc.gpsimd.memset(bias_q2[:], -0.2 * SHIFT)
        bias_e = cp.tile([128, 1], f32)
        nc.gpsimd.memset(bias_e[:], -0.8 * SHIFT)

        # ---- input DMAs, all on the sync engine: xht first (the k_bcast
        # matmul chain and the first masks depend on it), then xfp quarters
        # so the q -> eq -> W chunk pipeline starts on the first quarter.
        xht = sp.tile([64, HALF], bf16)
        nc.sync.dma_start(out=xht[:], in_=xht_d[:])
        wkb = sp.tile([64, 128], bf16)
        nc.sync.dma_start(out=wkb[:], in_=wkb_d[:])
        xfp = sp.tile([128, NCH, F], f16)
        nc.sync.dma_start(
            out=xfp[:, 0:8, :].rearrange("p c f -> p (c f)"),
            in_=xfp_d[:, 0:8 * F])
        wqkh = sp.tile([128, 128], f16)
        nc.sync.dma_start(out=wqkh[:], in_=wqkh_d[:])
        for g in range(1, 4):
            nc.sync.dma_start(
                out=xfp[:, g * 8:(g + 1) * 8, :].rearrange("p c f -> p (c f)"),
                in_=xfp_d[:, g * 8 * F:(g + 1) * 8 * F])
        xhp = sp.tile([128, NIC, F], f16)   # DMA issued later, gated on eq

        # ---- persistent sbuf ----
        q = sp.tile([128, NCH], f32)
        negq = sp.tile([128, NCH], f32)
        kk = sp.tile([128, NIC], f32)
        eq = sp.tile([128, NCH], f32)
        eq2 = sp.tile([128, NCH], f32)
        e = sp.tile([128, NIC], f32)
        k_bcast = sp.tile([128, HALF], f16)
        W = sp.tile([128, NCH, 130], f16)   # [U|u|V|v] * exp(-S) scaling
        C1 = sp.tile([128, NIC, 65], f32)
        C_all = sp.tile([128, NIC, 65], f32)
        rr = sp.tile([128, NIC], f32)
        o_sb = sp.tile([128, NIC, F], f32)
        gam = sp.tile([1, 130], f32)
        dlt = sp.tile([1, 130], f32)
        G16 = sp.tile([1, 130], f16)
        qprod = sp.tile([128, 8, F], f16)
        junkk = sp.tile([128, F], f16)

        # ---- k_bcast: one matmul stage, two 2-bank psum tiles so the
        # matmuls pipeline instead of serializing behind the copies.
        with tc.tile_pool(name="pre_ps", bufs=1, space="PSUM") as pp:
            kb = [pp.tile([128, 2, 512], f32, name=f"kb{i}") for i in range(2)]
            for g in range(4):
                nc.tensor.matmul(kb[g // 2][:, g % 2, :], wkb[:],
                                 xht[:, g * 512:(g + 1) * 512], start=True, stop=True)
            for g in range(2):
                nc.scalar.copy(
                    k_bcast[:, g * 1024:(g + 1) * 1024].rearrange(
                        "p (a b) -> p a b", a=2), kb[g][:])

        with tc.tile_pool(name="mmps", bufs=1, space="PSUM") as mps:
            mm = mps.tile([128, 7, 512], f32)
            gps = mps.tile([128, 512], f32)
            wqb = wqkh[:, 0:64]
            wkb2 = wqkh[:, 64:128]

            # ---- fused chunk-streamed pipeline, one quarter at a time.
            # Per quarter: eq/eq2, then per chunk: W scaling, mask, gamma and
            # the 16 accumulating matmuls — so the matmul stream starts as
            # soon as chunk 0 is ready and masks are produced just-in-time.
            # The NEXT quarter's q dots are woven between chunks so they
            # overlap this quarter's stream instead of gating it.
            # i-chunk ic -> bank ic % 7, column offset (ic // 7) * SLOT_OFF.
            wqb3 = dataclasses.replace(wqb, ap=[wqb.ap[0], [0, 8], wqb.ap[1]])

            def qdots(g):
                # q for one quarter: products then a per-chunk free-dim reduce
                cs = slice(g * 8, (g + 1) * 8)
                nc.vector.tensor_tensor(qprod[:], xfp[:, cs, :], wqb3, Alu.mult)
                nc.vector.tensor_reduce(q[:, cs], qprod[:], mybir.AxisListType.X,
                                        Alu.add)

            qdots(0)
            # keep-warm: junk matmuls into an unused gps region (cleared by
            # gamma c=0's start=True before any real use), hanging off the
            # pre-chain products and then a dense burst on k_bcast, so the PE
            # HAM clock gate is already at full rate when the stream begins.
            # start=False only touches clear-bit columns.
            nc.tensor.matmul(gps[0:1, 436:500], ones_col16[:], qprod[:, 0, 0:64],
                             start=False, stop=True, skip_group_check=True)
            for j in range(24):
                nc.tensor.matmul(gps[0:1, 436:500], ones_col16[:],
                                 k_bcast[:, j * 64:(j + 1) * 64],
                                 start=False, stop=True, skip_group_check=True)

            def prep_quarter(g):
                cs = slice(g * 8, (g + 1) * 8)
                nc.vector.tensor_scalar(negq[:, cs], q[:, cs], -1.0, None, Alu.mult)
                nc.scalar.activation(eq[:, cs], q[:, cs], Act.Exp, bias=bias_q[:])
                nc.scalar.activation(eq2[:, cs], q[:, cs], Act.Exp,
                                     bias=bias_q2[:], scale=0.2)
                nc.vector.tensor_copy(
                    W[:, cs, 64:65], eq[:, cs].rearrange("p (c o) -> p c o", o=1))
                nc.vector.tensor_copy(
                    W[:, cs, 129:130], eq2[:, cs].rearrange("p (c o) -> p c o", o=1))
                if g == 1:
                    # gate xhp (k dots feed only the post phase) behind eq so
                    # the scheduler cannot hoist the k dots into the q->eq->W
                    # critical window; landing early in quarter 2 is fine.
                    nc.vector.tensor_copy(xhp[0:1, 0, 0:1], eq[0:1, 15:16])
                    nc.gpsimd.dma_start(
                        out=xhp[:, :, :].rearrange("p c f -> p (c f)"), in_=xhp_d[:])

            def prep_chunk(c):
                # W scaling + mask + gamma for chunk c, emitted two chunks
                # ahead of c's matmuls so the mask LDWEIGHTS prefetches behind
                # the previous chunks' matmuls instead of serializing.
                if c % 8 == 0:
                    prep_quarter(c // 8)
                nc.vector.tensor_scalar(W[:, c, 0:64], xfp[:, c, :],
                                        eq[:, c:c + 1], None, Alu.mult)
                nc.vector.tensor_scalar(W[:, c, 65:129], xfp[:, c, :],
                                        eq2[:, c:c + 1], None, Alu.mult)
                m = mp.tile([128, HALF], f16, tag="mask", name=f"m{c}")
                if c in ACTSET:
                    nc.scalar.activation(m[:], k_bcast[:], Act.Sign,
                                         bias=q[:, c:c + 1])
                else:
                    nc.vector.tensor_scalar(m[:], k_bcast[:], negq[:, c:c + 1],
                                            2.0, Alu.is_gt, Alu.mult)
                goff = 0 if c in ACTSET else SLOT_OFF
                last = (c == LAST_ACT) if c in ACTSET else (c == NCH - 1)
                nc.tensor.matmul(gps[0:1, goff:goff + 130], ones_col16[:],
                                 W[:, c, :], start=(c == 0), stop=last,
                                 skip_group_check=True)
                return m

            masks = {}
            masks[0] = prep_chunk(0)
            masks[1] = prep_chunk(1)
            for c in range(NCH):
                if c + 2 < NCH:
                    masks[c + 2] = prep_chunk(c + 2)
                m = masks.pop(c)
                if c % 8 == 0 and c // 8 < 3:
                    qdots(c // 8 + 1)
                if NIC <= c < 2 * NIC:  # k dots fill DVE bubbles late
                    nc.vector.scalar_tensor_tensor(
                        junkk[:], xhp[:, c - NIC, :], 1.0, wkb2,
                        Alu.mult, Alu.mult, accum_out=kk[:, c - NIC:c - NIC + 1])
                if c == 2 * NIC - 1:
                    nc.scalar.activation(e[:], kk[:], Act.Exp,
                                         bias=bias_e[:], scale=-0.8)
                for ic in range(NIC):
                    bank, slot = ic % 7, ic // 7
                    off = slot * SLOT_OFF
                    nc.tensor.matmul(mm[:, bank, off:off + 130],
                                     m[:, ic * 128:(ic + 1) * 128],
                                     W[:, c, :],
                                     start=(c == 0 and slot == 0),
                                     stop=False,
                                     skip_group_check=True)

            # G correction: G_U = gam_U ; G_V = -2*dlt_V - gam_V.  Instead of
            # evacuating-and-adding, inject G straight into every PSUM
            # accumulator with tiny K=1 matmuls (ones_row16.T @ G16), then
            # compute the post phase directly from PSUM.
            nc.vector.tensor_copy(gam[:], gps[0:1, 0:130])
            nc.vector.tensor_copy(dlt[:], gps[0:1, SLOT_OFF:SLOT_OFF + 130])
            nc.vector.tensor_copy(G16[:, 0:65], gam[:, 0:65])
            nc.vector.scalar_tensor_tensor(
                G16[:, 65:130], dlt[:, 65:130], -2.0, gam[:, 65:130],
                Alu.mult, Alu.subtract)
            for ic in range(NIC):
                bank, slot = ic % 7, ic // 7
                off = slot * SLOT_OFF
                nc.tensor.matmul(mm[:, bank, off:off + 130], ones_row16[:],
                                 G16[:], start=False, stop=True,
                                 skip_group_check=True)

            # ---- post, directly from PSUM, in three pieces so the out-DMAs
            # overlap compute: C = e*T_V - T_U ; rr = 1/C[:,64] ; out = C*rr
            def post(lo, hi, boff, eng):
                sl = slice(lo, hi)
                nsl = hi - lo
                e_s = e[:, sl].rearrange("p (c o) -> p c o", o=1)
                e_b = dataclasses.replace(e_s, ap=[e_s.ap[0], e_s.ap[1], [0, 65]])
                nc.vector.tensor_tensor(C1[:, sl, :],
                                        mm[:, 0:nsl, boff + 65:boff + 130],
                                        e_b, Alu.mult)
                nc.vector.tensor_tensor(C_all[:, sl, :], C1[:, sl, :],
                                        mm[:, 0:nsl, boff:boff + 65], Alu.subtract)
                nc.vector.reciprocal(rr[:, sl], C_all[:, sl, 64:65])
                rr_s = rr[:, sl].rearrange("p (c o) -> p c o", o=1)
                rr_b = dataclasses.replace(rr_s, ap=[rr_s.ap[0], rr_s.ap[1], [0, F]])
                nc.vector.tensor_tensor(o_sb[:, sl, :], C_all[:, sl, 0:64],
                                        rr_b, Alu.mult)
                eng.dma_start(out=out_d[:, lo * F:hi * F], in_=o_sb[:, sl, :])

            post(0, 7, 0, nc.sync)
            post(7, 14, SLOT_OFF, nc.scalar)
            post(14, 16, 2 * SLOT_OFF, nc.sync)


def build_program():
    if "nc" in _CACHE:
        return _CACHE["nc"]
    from concourse import bacc, tile

    nc = bacc.Bacc("TRN2", target_bir_lowering=False, debug=False,
                   enable_asserts=True, num_devices=8)
    with tile.TileContext(nc) as tc:
        _body(nc, tc)
    nc.compile()
    _CACHE["nc"] = nc
    return nc


def make_in_maps(x, weight_key, weight_query):
    x = np.ascontiguousarray(np.asarray(x, dtype=np.float32))
    wk = np.asarray(weight_key, dtype=np.float32).reshape(-1)
    wq = np.asarray(weight_query, dtype=np.float32).reshape(-1)
    import ml_dtypes
    wkb = np.ascontiguousarray(
        np.repeat(wk[:, None], 128, axis=1).astype(ml_dtypes.bfloat16))  # [64, 128]
    wqkh = np.ascontiguousarray(
        np.tile(np.concatenate([wq, wk])[None, :], (128, 1))).astype(np.float16)
    in_maps = []
    for core in range(8):
        b, h = divmod(core, 2)
        xb = x[b]                                    # [N, F]
        xh = xb[h * HALF:(h + 1) * HALF]             # [HALF, F]
        xfp = np.ascontiguousarray(
            xb.reshape(NCH, 128, F).transpose(1, 0, 2).reshape(128, NCH * F)
        ).astype(np.float16)
        xhp = np.ascontiguousarray(
            xh.reshape(NIC, 128, F).transpose(1, 0, 2).reshape(128, NIC * F)
        ).astype(np.float16)
        in_maps.append({
            "xfp": xfp,
            "xhp": xhp,
            "xht": np.ascontiguousarray(xh.T.astype(ml_dtypes.bfloat16)),  # [64, HALF]
            "wkb": wkb,
            "wqkh": wqkh,
        })
    return in_maps


def assemble(results):
    out = np.empty((BZ, F, N), dtype=np.float32)
    for core in range(8):
        b, h = divmod(core, 2)
        o = results[core]["out"].reshape(128, NIC, F)        # [p, ic, f]
        # i_local = ic*128 + p  ->  [f, ic, p] then flatten
        out[b, :, h * HALF:(h + 1) * HALF] = o.transpose(2, 1, 0).reshape(F, HALF)
    return out


def kernel(x, weight_key, weight_query, _trace=False, _tmpdir=None):
    from concourse.bass_utils import run_bass_kernel_spmd

    nc = build_program()
    in_maps = make_in_maps(x, weight_key, weight_query)
    res = run_bass_kernel_spmd(nc, in_maps, core_ids=list(range(8)), trace=_trace,
                               tmpdir=_tmpdir)
    out = assemble(res.results)
    if _trace:
        return out, res
    return out


# revision 39
# speedup vs baseline: 1.2115x; 1.2115x over previous
"""GAT additive-attention kernel (nn_GAT) for 8 Trainium2 NeuronCores.

reference:
    k = x @ w_k; q = x @ w_q                      # [bz, N, 1]
    s[b,i,j]   = leaky_relu(k[b,i] + q[b,j], 0.2)
    attn       = softmax(s, axis=j)
    out        = (attn @ x).transpose(0, 2, 1)    # [bz, F, N]

Key identity: with sigma[i,j] = sign(k_i + q_j),
    exp(lrelu(s)) = exp(k_i)exp(q_j)       if s > 0
                  = exp(.2 k_i)exp(.2 q_j) if s <= 0
so with U = exp(q_j - S)*[x_j|1], V = exp(.2(q_j - S))*[x_j|1]:
    out_i = num_i / den_i,  [num|den]_i = (M@U)_i + e_i * (colsum(V) - (M@V))_i
where M = (sigma+1)/2 and e_i = exp(-.8 (k_i + S)).  Everything reduces
to ONE N x N masked matmul  T = sigma-ish @ [U|V]  (fp16) plus O(N*F)
pre/post work.

v5: the exponent shift S is a FIXED constant (fp16 is floating point, so
any S with max(q)-S <= ~10 keeps the top softmax weights in range and only
flushes weights ~e^-17 below the row top — far below the error budget).
This removes the global max(q) reduction from the critical path entirely:
q, exp(q), W and the gamma column sums all stream chunk-by-chunk behind
the x DMA, so the masked-matmul stream starts as soon as the first x
quarter lands.  k_bcast comes from one matmul stage (wk replicated as the
stationary operand against x^T) into four SEPARATE psum tiles (a shared
tile serializes matmul g+1 behind the evacuation of g via a whole-tile
WAR hazard).  The k dot-products (only needed for e in the post phase)
are gated behind eq via a dummy write so the scheduler cannot hoist them
into the critical window.  The G correction add is folded into the PSUM
evacuation, and the post phase runs in two pieces so the first out-DMA
overlaps the second piece's compute.

All 16 row-chunk accumulators live in PSUM simultaneously by packing
2-3 accumulation groups per bank: start=True (which clears the WHOLE
bank's has_written bits) is only used by the first group touching each
bank; later groups open with start=False, which overwrites where the
bits are clear and accumulates afterwards.

Sharding: core c handles batch b = c//2, row-half h = c%2 (2048 rows),
fully data-parallel (no collectives).
"""

import sys
import numpy as np

for _p in ("/opt/trn_rl_repo",):
    if _p not in sys.path:
        sys.path.insert(0, _p)

N = 4096
F = 64
BZ = 4
HALF = 2048
NCH = 32          # j-chunks of 128 (full N)
NIC = 16          # i-chunks of 128 (this core's half)
NEG_SLOPE = 0.2
SLOT_OFF = 176    # fp32 elems between accumulation groups within a bank
SHIFT = 40.0      # fixed exponent shift; safe while max(x @ w_q) < 50

# sign-mask chunks on the ACT engine (slot A).  The first chunks stay on the
# DVE so the matmul stream can start as soon as W chunk 0 exists; ACT gets 8
# mid-pattern chunks, which is all it can sustain alongside its other work.
ACTSET = frozenset(c for c in range(NCH) if c % 8 in (3, 4, 5))
LAST_ACT = max(ACTSET)

_CACHE = {}


def _body(nc, tc):
    import dataclasses
    import concourse.mybir as mybir

    f32 = mybir.dt.float32
    f16 = mybir.dt.float16
    bf16 = mybir.dt.bfloat16
    Alu = mybir.AluOpType
    Act = mybir.ActivationFunctionType

    # host-prepacked inputs (see make_in_maps); x ships as f16 to halve the
    # input DMA bytes (W is built in f16 anyway; q/k dots accumulate in fp32)
    xfp_d = nc.dram_tensor("xfp", [128, NCH * F], f16, kind="ExternalInput").ap()
    xhp_d = nc.dram_tensor("xhp", [128, NIC * F], f16, kind="ExternalInput").ap()
    xht_d = nc.dram_tensor("xht", [64, HALF], bf16, kind="ExternalInput").ap()
    wkb_d = nc.dram_tensor("wkb", [64, 128], bf16, kind="ExternalInput").ap()
    wqkh_d = nc.dram_tensor("wqkh", [128, 128], f16, kind="ExternalInput").ap()
    out_d = nc.dram_tensor("out", [128, NIC * F], f32, kind="ExternalOutput").ap()

    with (
        tc.tile_pool(name="const", bufs=1) as cp,
        tc.tile_pool(name="sb", bufs=1) as sp,
        tc.tile_pool(name="maskp", bufs=8) as mp,
    ):
        ones_row16 = cp.tile([1, 128], f16)
        nc.gpsimd.memset(ones_row16[:], 1.0)
        ones_col16 = cp.tile([128, 1], f16)
        nc.gpsimd.memset(ones_col16[:], 1.0)
        bias_q = cp.tile([128, 1], f32)
        nc.gpsimd.memset(bias_q[:], -SHIFT)
        bias_q2 = cp.tile([128, 1], f32)
        nc.gpsimd.memset(bias_q2[:], -0.2 * SHIFT)
        bias_e = cp.tile([128, 1], f32)
        nc.gpsimd.memset(bias_e[:], -0.8 * SHIFT)

        # ---- input DMAs, all on the sync engine: xht first (the k_bcast
        # matmul chain and the first masks depend on it), then xfp quarters
        # so the q -> eq -> W chunk pipeline starts on the first quarter.
        xht = sp.tile([64, HALF], bf16)
        nc.sync.dma_start(out=xht[:], in_=xht_d[:])
        wkb = sp.tile([64, 128], bf16)
        nc.sync.dma_start(out=wkb[:], in_=wkb_d[:])
        xfp = sp.tile([128, NCH, F], f16)
        nc.sync.dma_start(
            out=xfp[:, 0:8, :].rearrange("p c f -> p (c f)"),
            in_=xfp_d[:, 0:8 * F])
        wqkh = sp.tile([128, 128], f16)
        nc.sync.dma_start(out=wqkh[:], in_=wqkh_d[:])
        for g in range(1, 4):
            nc.sync.dma_start(
                out=xfp[:, g * 8:(g + 1) * 8, :].rearrange("p c f -> p (c f)"),
                in_=xfp_d[:, g * 8 * F:(g + 1) * 8 * F])
        xhp = sp.tile([128, NIC, F], f16)   # DMA issued later, gated on eq

        # ---- persistent sbuf ----
        q = sp.tile([128, NCH], f32)
        negq = sp.tile([128, NCH], f32)
        kk = sp.tile([128, NIC], f32)
        eq = sp.tile([128, NCH], f32)
        eq2 = sp.tile([128, NCH], f32)
        e = sp.tile([128, NIC], f32)
        k_bcast = sp.tile([128, HALF], f16)
        W = sp.tile([128, NCH, 130], f16)   # [U|u|V|v] * exp(-S) scaling
        C1 = sp.tile([128, NIC, 65], f32)
        C_all = sp.tile([128, NIC, 65], f32)
        rr = sp.tile([128, NIC], f32)
        o_sb = sp.tile([128, NIC, F], f32)
        gam = sp.tile([1, 130], f32)
        dlt = sp.tile([1, 130], f32)
        G16 = sp.tile([1, 130], f16)
        qprod = sp.tile([128, 8, F], f16)
        junkk = sp.tile([128, F], f16)

        # ---- k_bcast: one matmul stage, two 2-bank psum tiles so the
        # matmuls pipeline instead of serializing behind the copies.
        with tc.tile_pool(name="pre_ps", bufs=1, space="PSUM") as pp:
            kb = [pp.tile([128, 2, 512], f32, name=f"kb{i}") for i in range(2)]
            for g in range(4):
                nc.tensor.matmul(kb[g // 2][:, g % 2, :], wkb[:],
                                 xht[:, g * 512:(g + 1) * 512], start=True, stop=True)
            for g in range(2):
                nc.scalar.copy(
                    k_bcast[:, g * 1024:(g + 1) * 1024].rearrange(
                        "p (a b) -> p a b", a=2), kb[g][:])

        with tc.tile_pool(name="mmps", bufs=1, space="PSUM") as mps:
            mm = mps.tile([128, 7, 512], f32)
            gps = mps.tile([128, 512], f32)
            wqb = wqkh[:, 0:64]
            wkb2 = wqkh[:, 64:128]

            # ---- fused chunk-streamed pipeline, one quarter at a time.
            # Per quarter: eq/eq2, then per chunk: W scaling, mask, gamma and
            # the 16 accumulating matmuls — so the matmul stream starts as
            # soon as chunk 0 is ready and masks are produced just-in-time.
            # The NEXT quarter's q dots are woven between chunks so they
            # overlap this quarter's stream instead of gating it.
            # i-chunk ic -> bank ic % 7, column offset (ic // 7) * SLOT_OFF.
            wqb3 = dataclasses.replace(wqb, ap=[wqb.ap[0], [0, 8], wqb.ap[1]])

            def qdots(g):
                # q for one quarter: products then a per-chunk free-dim reduce
                cs = slice(g * 8, (g + 1) * 8)
                nc.vector.tensor_tensor(qprod[:], xfp[:, cs, :], wqb3, Alu.mult)
                nc.vector.tensor_reduce(q[:, cs], qprod[:], mybir.AxisListType.X,
                                        Alu.add)

            qdots(0)
            # keep-warm: junk matmuls into an unused gps region (cleared by
            # gamma c=0's start=True before any real use), each hanging off a
            # successively later pre-chain product so the PE HAM window never
            # sees a >3.4us idle gap between the k_bcast matmuls and the
            # stream.  start=False only touches clear-bit columns.
            nc.tensor.matmul(gps[0:1, 436:500], ones_col16[:], k_bcast[:, 0:64],
                             start=False, stop=True, skip_group_check=True)
            nc.tensor.matmul(gps[0:1, 436:500], ones_col16[:], qprod[:, 0, 0:64],
                             start=False, stop=True, skip_group_check=True)
            nc.tensor.matmul(gps[0:1, 436:500], ones_col16[:],
                             k_bcast[:, 1024:1088],
                             start=False, stop=True, skip_group_check=True)
            for g in range(4):
                cs = slice(g * 8, (g + 1) * 8)
                nc.vector.tensor_scalar(negq[:, cs], q[:, cs], -1.0, None, Alu.mult)
                nc.scalar.activation(eq[:, cs], q[:, cs], Act.Exp, bias=bias_q[:])
                nc.scalar.activation(eq2[:, cs], q[:, cs], Act.Exp,
                                     bias=bias_q2[:], scale=0.2)
                nc.vector.tensor_copy(
                    W[:, cs, 64:65], eq[:, cs].rearrange("p (c o) -> p c o", o=1))
                nc.vector.tensor_copy(
                    W[:, cs, 129:130], eq2[:, cs].rearrange("p (c o) -> p c o", o=1))
                if g == 1:
                    # gate xhp (k dots feed only the post phase) behind eq so
                    # the scheduler cannot hoist the k dots into the q->eq->W
                    # critical window; landing mid-quarter-2 is early enough.
                    nc.vector.tensor_copy(xhp[0:1, 0, 0:1], eq[0:1, 15:16])
                    nc.gpsimd.dma_start(
                        out=xhp[:, :, :].rearrange("p c f -> p (c f)"), in_=xhp_d[:])
                for c in range(g * 8, (g + 1) * 8):
                    nc.vector.tensor_scalar(W[:, c, 0:64], xfp[:, c, :],
                                            eq[:, c:c + 1], None, Alu.mult)
                    nc.vector.tensor_scalar(W[:, c, 65:129], xfp[:, c, :],
                                            eq2[:, c:c + 1], None, Alu.mult)
                    m = mp.tile([128, HALF], f16, tag="mask")
                    if c in ACTSET:
                        nc.scalar.activation(m[:], k_bcast[:], Act.Sign,
                                             bias=q[:, c:c + 1])
                    else:
                        nc.vector.tensor_scalar(m[:], k_bcast[:], negq[:, c:c + 1],
                                                2.0, Alu.is_gt, Alu.mult)
                    goff = 0 if c in ACTSET else SLOT_OFF
                    last = (c == LAST_ACT) if c in ACTSET else (c == NCH - 1)
                    nc.tensor.matmul(gps[0:1, goff:goff + 130], ones_col16[:],
                                     W[:, c, :], start=(c == 0), stop=last,
                                     skip_group_check=True)
                    if c % 8 == 0 and g < 3:
                        qdots(g + 1)
                    if NIC <= c < 2 * NIC:  # k dots fill DVE bubbles late
                        nc.vector.scalar_tensor_tensor(
                            junkk[:], xhp[:, c - NIC, :], 1.0, wkb2,
                            Alu.mult, Alu.mult, accum_out=kk[:, c - NIC:c - NIC + 1])
                    if c == 2 * NIC - 1:
                        nc.scalar.activation(e[:], kk[:], Act.Exp,
                                             bias=bias_e[:], scale=-0.8)
                    for ic in range(NIC):
                        bank, slot = ic % 7, ic // 7
                        off = slot * SLOT_OFF
                        nc.tensor.matmul(mm[:, bank, off:off + 130],
                                         m[:, ic * 128:(ic + 1) * 128],
                                         W[:, c, :],
                                         start=(c == 0 and slot == 0),
                                         stop=False,
                                         skip_group_check=True)

            # G correction: G_U = gam_U ; G_V = -2*dlt_V - gam_V.  Instead of
            # evacuating-and-adding, inject G straight into every PSUM
            # accumulator with tiny K=1 matmuls (ones_row16.T @ G16), then
            # compute the post phase directly from PSUM.
            nc.vector.tensor_copy(gam[:], gps[0:1, 0:130])
            nc.vector.tensor_copy(dlt[:], gps[0:1, SLOT_OFF:SLOT_OFF + 130])
            nc.vector.tensor_copy(G16[:, 0:65], gam[:, 0:65])
            nc.vector.scalar_tensor_tensor(
                G16[:, 65:130], dlt[:, 65:130], -2.0, gam[:, 65:130],
                Alu.mult, Alu.subtract)
            for ic in range(NIC):
                bank, slot = ic % 7, ic // 7
                off = slot * SLOT_OFF
                nc.tensor.matmul(mm[:, bank, off:off + 130], ones_row16[:],
                                 G16[:], start=False, stop=True,
                                 skip_group_check=True)

            # ---- post, directly from PSUM, in three pieces so the out-DMAs
            # overlap compute: C = e*T_V - T_U ; rr = 1/C[:,64] ; out = C*rr
            def post(lo, hi, boff, eng):
                sl = slice(lo, hi)
                nsl = hi - lo
                e_s = e[:, sl].rearrange("p (c o) -> p c o", o=1)
                e_b = dataclasses.replace(e_s, ap=[e_s.ap[0], e_s.ap[1], [0, 65]])
                nc.vector.tensor_tensor(C1[:, sl, :],
                                        mm[:, 0:nsl, boff + 65:boff + 130],
                                        e_b, Alu.mult)
                nc.vector.tensor_tensor(C_all[:, sl, :], C1[:, sl, :],
                                        mm[:, 0:nsl, boff:boff + 65], Alu.subtract)
                nc.vector.reciprocal(rr[:, sl], C_all[:, sl, 64:65])
                rr_s = rr[:, sl].rearrange("p (c o) -> p c o", o=1)
                rr_b = dataclasses.replace(rr_s, ap=[rr_s.ap[0], rr_s.ap[1], [0, F]])
                nc.vector.tensor_tensor(o_sb[:, sl, :], C_all[:, sl, 0:64],
                                        rr_b, Alu.mult)
                eng.dma_start(out=out_d[:, lo * F:hi * F], in_=o_sb[:, sl, :])

            post(0, 7, 0, nc.sync)
            post(7, 14, SLOT_OFF, nc.scalar)
            post(14, 16, 2 * SLOT_OFF, nc.sync)


def build_program():
    if "nc" in _CACHE:
        return _CACHE["nc"]
    from concourse import bacc, tile

    nc = bacc.Bacc("TRN2", target_bir_lowering=False, debug=False,
                   enable_asserts=True, num_devices=8)
    with tile.TileContext(nc) as tc:
        _body(nc, tc)
    nc.compile()
    _CACHE["nc"] = nc
    return nc


def make_in_maps(x, weight_key, weight_query):
    x = np.ascontiguousarray(np.asarray(x, dtype=np.float32))
    wk = np.asarray(weight_key, dtype=np.float32).reshape(-1)
    wq = np.asarray(weight_query, dtype=np.float32).reshape(-1)
    import ml_dtypes
    wkb = np.ascontiguousarray(
        np.repeat(wk[:, None], 128, axis=1).astype(ml_dtypes.bfloat16))  # [64, 128]
    wqkh = np.ascontiguousarray(
        np.tile(np.concatenate([wq, wk])[None, :], (128, 1))).astype(np.float16)
    in_maps = []
    for core in range(8):
        b, h = divmod(core, 2)
        xb = x[b]                                    # [N, F]
        xh = xb[h * HALF:(h + 1) * HALF]             # [HALF, F]
        xfp = np.ascontiguousarray(
            xb.reshape(NCH, 128, F).transpose(1, 0, 2).reshape(128, NCH * F)
        ).astype(np.float16)
        xhp = np.ascontiguousarray(
            xh.reshape(NIC, 128, F).transpose(1, 0, 2).reshape(128, NIC * F)
        ).astype(np.float16)
        in_maps.append({
            "xfp": xfp,
            "xhp": xhp,
            "xht": np.ascontiguousarray(xh.T.astype(ml_dtypes.bfloat16)),  # [64, HALF]
            "wkb": wkb,
            "wqkh": wqkh,
        })
    return in_maps


def assemble(results):
    out = np.empty((BZ, F, N), dtype=np.float32)
    for core in range(8):
        b, h = divmod(core, 2)
        o = results[core]["out"].reshape(128, NIC, F)        # [p, ic, f]
        # i_local = ic*128 + p  ->  [f, ic, p] then flatten
        out[b, :, h * HALF:(h + 1) * HALF] = o.transpose(2, 1, 0).reshape(F, HALF)
    return out


def kernel(x, weight_key, weight_query, _trace=False, _tmpdir=None):
    from concourse.bass_utils import run_bass_kernel_spmd

    nc = build_program()
    in_maps = make_in_maps(x, weight_key, weight_query)
    res = run_bass_kernel_spmd(nc, in_maps, core_ids=list(range(8)), trace=_trace,
                               tmpdir=_tmpdir)
    out = assemble(res.results)
    if _trace:
        return out, res
    return out


# revision 41
# speedup vs baseline: 1.2120x; 1.0004x over previous
"""GAT additive-attention kernel (nn_GAT) for 8 Trainium2 NeuronCores.

reference:
    k = x @ w_k; q = x @ w_q                      # [bz, N, 1]
    s[b,i,j]   = leaky_relu(k[b,i] + q[b,j], 0.2)
    attn       = softmax(s, axis=j)
    out        = (attn @ x).transpose(0, 2, 1)    # [bz, F, N]

Key identity: with sigma[i,j] = sign(k_i + q_j),
    exp(lrelu(s)) = exp(k_i)exp(q_j)       if s > 0
                  = exp(.2 k_i)exp(.2 q_j) if s <= 0
so with U = exp(q_j - S)*[x_j|1], V = exp(.2(q_j - S))*[x_j|1]:
    out_i = num_i / den_i,  [num|den]_i = (M@U)_i + e_i * (colsum(V) - (M@V))_i
where M = (sigma+1)/2 and e_i = exp(-.8 (k_i + S)).  Everything reduces
to ONE N x N masked matmul  T = sigma-ish @ [U|V]  (fp16) plus O(N*F)
pre/post work.

v5: the exponent shift S is a FIXED constant (fp16 is floating point, so
any S with max(q)-S <= ~10 keeps the top softmax weights in range and only
flushes weights ~e^-17 below the row top — far below the error budget).
This removes the global max(q) reduction from the critical path entirely:
q, exp(q), W and the gamma column sums all stream chunk-by-chunk behind
the x DMA, so the masked-matmul stream starts as soon as the first x
quarter lands.  k_bcast comes from one matmul stage (wk replicated as the
stationary operand against x^T) into four SEPARATE psum tiles (a shared
tile serializes matmul g+1 behind the evacuation of g via a whole-tile
WAR hazard).  The k dot-products (only needed for e in the post phase)
are gated behind eq via a dummy write so the scheduler cannot hoist them
into the critical window.  The G correction add is folded into the PSUM
evacuation, and the post phase runs in two pieces so the first out-DMA
overlaps the second piece's compute.

All 16 row-chunk accumulators live in PSUM simultaneously by packing
2-3 accumulation groups per bank: start=True (which clears the WHOLE
bank's has_written bits) is only used by the first group touching each
bank; later groups open with start=False, which overwrites where the
bits are clear and accumulates afterwards.

Sharding: core c handles batch b = c//2, row-half h = c%2 (2048 rows),
fully data-parallel (no collectives).
"""

import sys
import numpy as np

for _p in ("/opt/trn_rl_repo",):
    if _p not in sys.path:
        sys.path.insert(0, _p)

N = 4096
F = 64
BZ = 4
HALF = 2048
NCH = 32          # j-chunks of 128 (full N)
NIC = 16          # i-chunks of 128 (this core's half)
NEG_SLOPE = 0.2
SLOT_OFF = 176    # fp32 elems between accumulation groups within a bank
SHIFT = 40.0      # fixed exponent shift; safe while max(x @ w_q) < 50

# sign-mask chunks on the ACT engine (slot A).  The first chunks stay on the
# DVE so the matmul stream can start as soon as W chunk 0 exists; ACT gets 8
# mid-pattern chunks, which is all it can sustain alongside its other work.
ACTSET = frozenset(c for c in range(NCH) if c % 8 in (3, 4, 5))
LAST_ACT = max(ACTSET)

_CACHE = {}


def _body(nc, tc):
    import dataclasses
    import concourse.mybir as mybir

    f32 = mybir.dt.float32
    f16 = mybir.dt.float16
    bf16 = mybir.dt.bfloat16
    Alu = mybir.AluOpType
    Act = mybir.ActivationFunctionType

    # host-prepacked inputs (see make_in_maps); x ships as f16 to halve the
    # input DMA bytes (W is built in f16 anyway; q/k dots accumulate in fp32)
    xfp_d = nc.dram_tensor("xfp", [128, NCH * F], f16, kind="ExternalInput").ap()
    xhp_d = nc.dram_tensor("xhp", [128, NIC * F], f16, kind="ExternalInput").ap()
    xht_d = nc.dram_tensor("xht", [64, HALF], bf16, kind="ExternalInput").ap()
    wkb_d = nc.dram_tensor("wkb", [64, 128], bf16, kind="ExternalInput").ap()
    wqkh_d = nc.dram_tensor("wqkh", [128, 128], f16, kind="ExternalInput").ap()
    out_d = nc.dram_tensor("out", [128, NIC * F], f32, kind="ExternalOutput").ap()

    with (
        tc.tile_pool(name="const", bufs=1) as cp,
        tc.tile_pool(name="sb", bufs=1) as sp,
        tc.tile_pool(name="maskp", bufs=8) as mp,
    ):
        ones_row16 = cp.tile([1, 128], f16)
        nc.gpsimd.memset(ones_row16[:], 1.0)
        ones_col16 = cp.tile([128, 1], f16)
        nc.gpsimd.memset(ones_col16[:], 1.0)
        bias_q = cp.tile([128, 1], f32)
        nc.gpsimd.memset(bias_q[:], -SHIFT)
        bias_q2 = cp.tile([128, 1], f32)
        nc.gpsimd.memset(bias_q2[:], -0.2 * SHIFT)
        bias_e = cp.tile([128, 1], f32)
        nc.gpsimd.memset(bias_e[:], -0.8 * SHIFT)

        # ---- input DMAs, all on the sync engine: xht first (the k_bcast
        # matmul chain and the first masks depend on it), then xfp quarters
        # so the q -> eq -> W chunk pipeline starts on the first quarter.
        xht = sp.tile([64, HALF], bf16)
        nc.sync.dma_start(out=xht[:], in_=xht_d[:])
        wkb = sp.tile([64, 128], bf16)
        nc.sync.dma_start(out=wkb[:], in_=wkb_d[:])
        xfp = sp.tile([128, NCH, F], f16)
        nc.sync.dma_start(
            out=xfp[:, 0:8, :].rearrange("p c f -> p (c f)"),
            in_=xfp_d[:, 0:8 * F])
        wqkh = sp.tile([128, 128], f16)
        nc.sync.dma_start(out=wqkh[:], in_=wqkh_d[:])
        # quarters 1-3 as one 3KB-row call: same bytes, a third the DMA
        # descriptors (the quarter calls were descriptor-rate bound)
        nc.sync.dma_start(
            out=xfp[:, 8:32, :].rearrange("p c f -> p (c f)"),
            in_=xfp_d[:, 8 * F:32 * F])
        xhp = sp.tile([128, NIC, F], f16)   # DMA issued later, gated on eq

        # ---- persistent sbuf ----
        q = sp.tile([128, NCH], f32)
        negq = sp.tile([128, NCH], f32)
        kk = sp.tile([128, NIC], f32)
        eq = sp.tile([128, NCH], f32)
        eq2 = sp.tile([128, NCH], f32)
        e = sp.tile([128, NIC], f32)
        k_bcast = sp.tile([128, HALF], f16)
        W = sp.tile([128, NCH, 130], f16)   # [U|u|V|v] * exp(-S) scaling
        C1 = sp.tile([128, NIC, 65], f32)
        C_all = sp.tile([128, NIC, 65], f32)
        rr = sp.tile([128, NIC], f32)
        o_sb = sp.tile([128, NIC, F], f32)
        gam = sp.tile([1, 130], f32)
        dlt = sp.tile([1, 130], f32)
        G16 = sp.tile([1, 130], f16)
        qprod = sp.tile([128, 8, F], f16)
        junkk = sp.tile([128, F], f16)

        # ---- k_bcast: one matmul stage, two 2-bank psum tiles so the
        # matmuls pipeline instead of serializing behind the copies.
        with tc.tile_pool(name="pre_ps", bufs=1, space="PSUM") as pp:
            kb = [pp.tile([128, 2, 512], f32, name=f"kb{i}") for i in range(2)]
            for g in range(4):
                nc.tensor.matmul(kb[g // 2][:, g % 2, :], wkb[:],
                                 xht[:, g * 512:(g + 1) * 512], start=True, stop=True)
            for g in range(2):
                nc.scalar.copy(
                    k_bcast[:, g * 1024:(g + 1) * 1024].rearrange(
                        "p (a b) -> p a b", a=2), kb[g][:])

        with tc.tile_pool(name="mmps", bufs=1, space="PSUM") as mps:
            mm = mps.tile([128, 7, 512], f32)
            gps = mps.tile([128, 512], f32)
            wqb = wqkh[:, 0:64]
            wkb2 = wqkh[:, 64:128]

            # ---- fused chunk-streamed pipeline, one quarter at a time.
            # Per quarter: eq/eq2, then per chunk: W scaling, mask, gamma and
            # the 16 accumulating matmuls — so the matmul stream starts as
            # soon as chunk 0 is ready and masks are produced just-in-time.
            # The NEXT quarter's q dots are woven between chunks so they
            # overlap this quarter's stream instead of gating it.
            # i-chunk ic -> bank ic % 7, column offset (ic // 7) * SLOT_OFF.
            wqb3 = dataclasses.replace(wqb, ap=[wqb.ap[0], [0, 8], wqb.ap[1]])

            def qdots(g):
                # q for one quarter: products then a per-chunk free-dim reduce
                cs = slice(g * 8, (g + 1) * 8)
                nc.vector.tensor_tensor(qprod[:], xfp[:, cs, :], wqb3, Alu.mult)
                nc.vector.tensor_reduce(q[:, cs], qprod[:], mybir.AxisListType.X,
                                        Alu.add)

            qdots(0)
            # keep-warm: junk matmuls into an unused gps region (cleared by
            # gamma c=0's start=True before any real use), each hanging off a
            # successively later pre-chain product so the PE HAM window never
            # sees a >3.4us idle gap between the k_bcast matmuls and the
            # stream.  start=False only touches clear-bit columns.
            nc.tensor.matmul(gps[0:1, 436:500], ones_col16[:], k_bcast[:, 0:64],
                             start=False, stop=True, skip_group_check=True)
            nc.tensor.matmul(gps[0:1, 436:500], ones_col16[:], qprod[:, 0, 0:64],
                             start=False, stop=True, skip_group_check=True)
            for j in range(7):
                nc.tensor.matmul(gps[0:1, 436:500], ones_col16[:],
                                 k_bcast[:, 1024 + j * 64:1088 + j * 64],
                                 start=False, stop=True, skip_group_check=True)
            for g in range(4):
                cs = slice(g * 8, (g + 1) * 8)
                nc.vector.tensor_scalar(negq[:, cs], q[:, cs], -1.0, None, Alu.mult)
                nc.scalar.activation(eq[:, cs], q[:, cs], Act.Exp, bias=bias_q[:])
                nc.scalar.activation(eq2[:, cs], q[:, cs], Act.Exp,
                                     bias=bias_q2[:], scale=0.2)
                nc.vector.tensor_copy(
                    W[:, cs, 64:65], eq[:, cs].rearrange("p (c o) -> p c o", o=1))
                nc.vector.tensor_copy(
                    W[:, cs, 129:130], eq2[:, cs].rearrange("p (c o) -> p c o", o=1))
                if g == 1:
                    # gate xhp (k dots feed only the post phase) behind eq so
                    # the scheduler cannot hoist the k dots into the q->eq->W
                    # critical window; landing mid-quarter-2 is early enough.
                    nc.vector.tensor_copy(xhp[0:1, 0, 0:1], eq[0:1, 15:16])
                    nc.gpsimd.dma_start(
                        out=xhp[:, :, :].rearrange("p c f -> p (c f)"), in_=xhp_d[:])
                for c in range(g * 8, (g + 1) * 8):
                    nc.vector.tensor_scalar(W[:, c, 0:64], xfp[:, c, :],
                                            eq[:, c:c + 1], None, Alu.mult)
                    nc.vector.tensor_scalar(W[:, c, 65:129], xfp[:, c, :],
                                            eq2[:, c:c + 1], None, Alu.mult)
                    m = mp.tile([128, HALF], f16, tag="mask")
                    if c in ACTSET:
                        nc.scalar.activation(m[:], k_bcast[:], Act.Sign,
                                             bias=q[:, c:c + 1])
                    else:
                        nc.vector.tensor_scalar(m[:], k_bcast[:], negq[:, c:c + 1],
                                                2.0, Alu.is_gt, Alu.mult)
                    goff = 0 if c in ACTSET else SLOT_OFF
                    last = (c == LAST_ACT) if c in ACTSET else (c == NCH - 1)
                    nc.tensor.matmul(gps[0:1, goff:goff + 130], ones_col16[:],
                                     W[:, c, :], start=(c == 0), stop=last,
                                     skip_group_check=True)
                    if c % 8 == 0 and g < 3:
                        qdots(g + 1)
                    if NIC <= c < 2 * NIC:  # k dots fill DVE bubbles late
                        nc.vector.scalar_tensor_tensor(
                            junkk[:], xhp[:, c - NIC, :], 1.0, wkb2,
                            Alu.mult, Alu.mult, accum_out=kk[:, c - NIC:c - NIC + 1])
                    if c == 2 * NIC - 1:
                        nc.scalar.activation(e[:], kk[:], Act.Exp,
                                             bias=bias_e[:], scale=-0.8)
                    for ic in range(NIC):
                        bank, slot = ic % 7, ic // 7
                        off = slot * SLOT_OFF
                        nc.tensor.matmul(mm[:, bank, off:off + 130],
                                         m[:, ic * 128:(ic + 1) * 128],
                                         W[:, c, :],
                                         start=(c == 0 and slot == 0),
                                         stop=False,
                                         skip_group_check=True)

            # G correction: G_U = gam_U ; G_V = -2*dlt_V - gam_V.  Instead of
            # evacuating-and-adding, inject G straight into every PSUM
            # accumulator with tiny K=1 matmuls (ones_row16.T @ G16), then
            # compute the post phase directly from PSUM.
            nc.vector.tensor_copy(gam[:], gps[0:1, 0:130])
            nc.vector.tensor_copy(dlt[:], gps[0:1, SLOT_OFF:SLOT_OFF + 130])
            nc.vector.tensor_copy(G16[:, 0:65], gam[:, 0:65])
            nc.vector.scalar_tensor_tensor(
                G16[:, 65:130], dlt[:, 65:130], -2.0, gam[:, 65:130],
                Alu.mult, Alu.subtract)
            for ic in range(NIC):
                bank, slot = ic % 7, ic // 7
                off = slot * SLOT_OFF
                nc.tensor.matmul(mm[:, bank, off:off + 130], ones_row16[:],
                                 G16[:], start=False, stop=True,
                                 skip_group_check=True)

            # ---- post, directly from PSUM, in three pieces so the out-DMAs
            # overlap compute: C = e*T_V - T_U ; rr = 1/C[:,64] ; out = C*rr
            def post(lo, hi, boff, eng):
                sl = slice(lo, hi)
                nsl = hi - lo
                e_s = e[:, sl].rearrange("p (c o) -> p c o", o=1)
                e_b = dataclasses.replace(e_s, ap=[e_s.ap[0], e_s.ap[1], [0, 65]])
                nc.vector.tensor_tensor(C1[:, sl, :],
                                        mm[:, 0:nsl, boff + 65:boff + 130],
                                        e_b, Alu.mult)
                nc.vector.tensor_tensor(C_all[:, sl, :], C1[:, sl, :],
                                        mm[:, 0:nsl, boff:boff + 65], Alu.subtract)
                nc.vector.reciprocal(rr[:, sl], C_all[:, sl, 64:65])
                rr_s = rr[:, sl].rearrange("p (c o) -> p c o", o=1)
                rr_b = dataclasses.replace(rr_s, ap=[rr_s.ap[0], rr_s.ap[1], [0, F]])
                nc.vector.tensor_tensor(o_sb[:, sl, :], C_all[:, sl, 0:64],
                                        rr_b, Alu.mult)
                eng.dma_start(out=out_d[:, lo * F:hi * F], in_=o_sb[:, sl, :])

            post(0, 7, 0, nc.sync)
            post(7, 14, SLOT_OFF, nc.scalar)
            post(14, 16, 2 * SLOT_OFF, nc.sync)


def build_program():
    if "nc" in _CACHE:
        return _CACHE["nc"]
    from concourse import bacc, tile

    nc = bacc.Bacc("TRN2", target_bir_lowering=False, debug=False,
                   enable_asserts=True, num_devices=8)
    with tile.TileContext(nc) as tc:
        _body(nc, tc)
    nc.compile()
    _CACHE["nc"] = nc
    return nc


def make_in_maps(x, weight_key, weight_query):
    x = np.ascontiguousarray(np.asarray(x, dtype=np.float32))
    wk = np.asarray(weight_key, dtype=np.float32).reshape(-1)
    wq = np.asarray(weight_query, dtype=np.float32).reshape(-1)
    import ml_dtypes
    wkb = np.ascontiguousarray(
        np.repeat(wk[:, None], 128, axis=1).astype(ml_dtypes.bfloat16))  # [64, 128]
    wqkh = np.ascontiguousarray(
        np.tile(np.concatenate([wq, wk])[None, :], (128, 1))).astype(np.float16)
    in_maps = []
    for core in range(8):
        b, h = divmod(core, 2)
        xb = x[b]                                    # [N, F]
        xh = xb[h * HALF:(h + 1) * HALF]             # [HALF, F]
        xfp = np.ascontiguousarray(
            xb.reshape(NCH, 128, F).transpose(1, 0, 2).reshape(128, NCH * F)
        ).astype(np.float16)
        xhp = np.ascontiguousarray(
            xh.reshape(NIC, 128, F).transpose(1, 0, 2).reshape(128, NIC * F)
        ).astype(np.float16)
        in_maps.append({
            "xfp": xfp,
            "xhp": xhp,
            "xht": np.ascontiguousarray(xh.T.astype(ml_dtypes.bfloat16)),  # [64, HALF]
            "wkb": wkb,
            "wqkh": wqkh,
        })
    return in_maps


def assemble(results):
    out = np.empty((BZ, F, N), dtype=np.float32)
    for core in range(8):
        b, h = divmod(core, 2)
        o = results[core]["out"].reshape(128, NIC, F)        # [p, ic, f]
        # i_local = ic*128 + p  ->  [f, ic, p] then flatten
        out[b, :, h * HALF:(h + 1) * HALF] = o.transpose(2, 1, 0).reshape(F, HALF)
    return out


def kernel(x, weight_key, weight_query, _trace=False, _tmpdir=None):
    from concourse.bass_utils import run_bass_kernel_spmd

    nc = build_program()
    in_maps = make_in_maps(x, weight_key, weight_query)
    res = run_bass_kernel_spmd(nc, in_maps, core_ids=list(range(8)), trace=_trace,
                               tmpdir=_tmpdir)
    out = assemble(res.results)
    if _trace:
        return out, res
    return out
